# revision 9
# baseline (speedup 1.0000x reference)
"""GAT multi-head attention kernel for Trainium2 (8 NeuronCores, data-parallel over batch).

Problem (hardcoded): h [8,1024,128] f32, W [8,128,16] f32, Wa [8,32] f32.
  g   = einsum('bni,hid->hbnd', h, W)
  e   = leakyrelu(g@a_src [i] + g@a_dst [j], 0.2)      # [H,B,N,N]
  P   = softmax_j(e)
  out = relu(einsum('hbij,hbjd->bihd', P, g)).reshape(B,N,H*hd)

Sharding: graph b -> core b. Each core computes all 8 heads for its graph.

Key algebraic trick: with s=g@a_src, t=g@a_dst,
  exp(leakyrelu(s_i+t_j)) = max(exp(s_i)*exp(t_j), exp(0.2 s_i)*exp(0.2 t_j))
so the NxN unnormalized probabilities are built from broadcast row/col vectors
with 2 vector-engine ops per tile (no NxN transcendentals).
The softmax denominator falls out of the same PE stream by appending a ones
column to g in the attention matmul ([g|1] stationary).
"""
import numpy as np
from contextlib import ExitStack

import concourse.bass as bass
import concourse.tile as tile
from concourse import bacc, mybir
from concourse import bass_utils

# ---- problem constants (from spec; kernel.py must be self-contained) ----
B, N, DI, H, HD = 8, 1024, 128, 8, 16
SLOPE = 0.2
NC128 = N // 128            # 8 chunks of 128
FP32 = mybir.dt.float32
BF16 = mybir.dt.bfloat16

AF = mybir.ActivationFunctionType
ALU = mybir.AluOpType

# dtype of the NxN probability tiles + attention-matmul operands
DT_MM = BF16


def build_nc(iters: int = 1):
    """Build + compile the per-core Bass program (same program on all cores)."""
    nc = bacc.Bacc("TRN2", target_bir_lowering=False, debug=False, num_devices=8)

    hb_d = nc.dram_tensor("hb", [N, DI], FP32, kind="ExternalInput")
    wall_d = nc.dram_tensor("wall", [DI, H * HD], FP32, kind="ExternalInput")
    wabd_d = nc.dram_tensor("wabd", [DI, 2 * H], FP32, kind="ExternalInput")
    ident_d = nc.dram_tensor("ident", [128, 128], FP32, kind="ExternalInput")
    ident4_d = nc.dram_tensor("ident4", [128, 32], FP32, kind="ExternalInput")
    out_d = nc.dram_tensor("out", [N, H * HD], FP32, kind="ExternalOutput")

    with tile.TileContext(nc) as tc:
        with ExitStack() as ctx:
            if iters > 1:
                fi = ctx.enter_context(tc.For_i(0, iters, 1))
            _body(ctx, tc, hb_d, wall_d, wabd_d, ident_d, ident4_d, out_d)
    nc.compile()
    return nc


def _body(ctx, tc, hb_d, wall_d, wabd_d, ident_d, ident4_d, out_d):
    nc = tc.nc
    consts = ctx.enter_context(tc.tile_pool(name="consts", bufs=1))
    sb = ctx.enter_context(tc.tile_pool(name="sb", bufs=3))
    ps_small = ctx.enter_context(tc.tile_pool(name="ps_small", bufs=2, space="PSUM"))
    ps_o4 = ctx.enter_context(tc.tile_pool(name="ps_o4", bufs=2, space="PSUM"))
    ps_t4 = ctx.enter_context(tc.tile_pool(name="ps_t4", bufs=2, space="PSUM"))

    # ---- constants in ----
    ident = consts.tile([128, 128], FP32)
    nc.sync.dma_start(ident[:], ident_d.ap())
    ident4 = consts.tile([128, 32], FP32)
    nc.sync.dma_start(ident4[:], ident4_d.ap())
    wall = consts.tile([128, H * HD], FP32)
    nc.sync.dma_start(wall[:], wall_d.ap())
    wabd = consts.tile([128, 2 * H], FP32)
    nc.sync.dma_start(wabd[:], wabd_d.ap())

    # ---- phase A: load h, transpose to hT [128 i, 1024 n] ----
    hT = consts.tile([128, N], FP32)
    for icn in range(NC128):
        hn = sb.tile([128, 128], FP32, tag="hn")
        nc.sync.dma_start(hn[:], hb_d.ap()[icn * 128:(icn + 1) * 128, :])
        pt = ps_small.tile([128, 128], FP32, tag="ps", padded_shape=[128, 512])
        nc.tensor.transpose(pt[:], hn[:], ident[:])
        nc.vector.tensor_copy(hT[:, icn * 128:(icn + 1) * 128], pt[:])

    # ---- phase B: gT [128 (h,d), 1024 n] = wall.T @ hT ----
    gT = consts.tile([128, N], FP32)
    for half in range(2):
        ps = ps_small.tile([128, 512], FP32, tag="ps", padded_shape=[128, 512])
        nc.tensor.matmul(ps[:], wall[:], hT[:, half * 512:(half + 1) * 512],
                         start=True, stop=True)
        nc.vector.tensor_copy(gT[:, half * 512:(half + 1) * 512], ps[:])

    # ---- phase C: g_ext [128 j, jc*136 + h*17 + (0..15)] = g, col 16 of each
    # 17-block = 1.0 (ones column for the softmax denominator) ----
    g_ext = consts.tile([128, NC128 * 136], DT_MM)
    nc.vector.memset(g_ext[:], 1.0)
    for jc in range(NC128):
        ps = ps_small.tile([128, 128], FP32, tag="ps", padded_shape=[128, 512])
        nc.tensor.matmul(ps[:], hT[:, jc * 128:(jc + 1) * 128], wall[:],
                         start=True, stop=True)
        dst = g_ext[:, jc * 136:(jc + 1) * 136].rearrange(
            "p (h q) -> p h q", q=17)[:, :, 0:16]
        src = ps[:].rearrange("p (h q) -> p h q", q=16)
        nc.vector.tensor_copy(dst, src)

    # ---- phase D: s/t vectors, exp rows/cols ----
    # st [128 n, jc*16 + (0..7 s_h | 8..15 t_h)]
    st = consts.tile([128, NC128 * 16], FP32)
    for jc in range(NC128):
        ps = ps_small.tile([128, 16], FP32, tag="ps", padded_shape=[128, 512])
        nc.tensor.matmul(ps[:], gT[:, jc * 128:(jc + 1) * 128], wabd[:],
                         start=True, stop=True)
        nc.vector.tensor_copy(st[:, jc * 16:(jc + 1) * 16], ps[:])

    # srows [8 h, 1024 i] = s_h(i)
    srows = consts.tile([8, N], FP32)
    for half in range(2):
        ps = ps_small.tile([8, 512], FP32, tag="ps", padded_shape=[128, 512])
        nc.tensor.matmul(ps[:], wabd[:, 0:8], gT[:, half * 512:(half + 1) * 512],
                         start=True, stop=True)
        nc.vector.tensor_copy(srows[:, half * 512:(half + 1) * 512], ps[:])

    # E rows: e1 = exp(s), e2 = exp(0.2 s)   [8, 1024] -> DT_MM
    e1rows = consts.tile([8, N], DT_MM)
    nc.scalar.activation(e1rows[:], srows[:], AF.Exp)
    e2rows = consts.tile([8, N], DT_MM)
    nc.scalar.activation(e2rows[:], srows[:], AF.Exp, scale=SLOPE)

    # F cols: f1[:, jc*8+h] = exp(t_h on chunk jc), f2 = exp(0.2 t)  (f32)
    t_view = st[:].rearrange("p (c q) -> p c q", q=16)[:, :, 8:16]
    f1 = consts.tile([128, NC128 * 8], FP32)
    nc.scalar.activation(f1[:].rearrange("p (c q) -> p c q", q=8), t_view, AF.Exp)
    f2 = consts.tile([128, NC128 * 8], FP32)
    nc.scalar.activation(f2[:].rearrange("p (c q) -> p c q", q=8), t_view, AF.Exp,
                         scale=SLOPE)

    # broadcast E rows across partitions: e1b/e2b [128, h*1024 + i]
    # (SBUF sources can't have zero partition step -> bounce via DRAM scratch)
    dram = ctx.enter_context(tc.tile_pool(name="dram", bufs=1, space="DRAM"))
    e1rows_d = dram.tile([H, N], DT_MM)
    nc.sync.dma_start(e1rows_d[:], e1rows[:])
    e2rows_d = dram.tile([H, N], DT_MM)
    nc.sync.dma_start(e2rows_d[:], e2rows[:])
    e1b = consts.tile([128, H * N], DT_MM)
    e2b = consts.tile([128, H * N], DT_MM)
    for h in range(H):
        nc.sync.dma_start(e1b[:, h * N:(h + 1) * N],
                          e1rows_d[h:h + 1, :].partition_broadcast(128))
        nc.sync.dma_start(e2b[:, h * N:(h + 1) * N],
                          e2rows_d[h:h + 1, :].partition_broadcast(128))

    # ---- phase E+F: per group of 4 heads ----
    outt = [consts.tile([128, H * HD], FP32, name=f"outt{i}") for i in range(NC128)]

    for grp in range(2):
        o4 = ps_o4.tile([128, N], FP32, tag="o4")  # 2 banks; 4 heads packed at
        # partition offsets 32*hh, rows 0..16 each (16 num + 1 den)
        for hh in range(4):
            h = grp * 4 + hh
            for jc in range(NC128):
                # unnormalized probs tile: [128 j, 1024 i]
                bt = sb.tile([128, N], DT_MM, tag="bt")
                nc.vector.tensor_scalar(
                    bt[:], e2b[:, h * N:(h + 1) * N],
                    f2[:, jc * 8 + h:jc * 8 + h + 1], None, ALU.mult)
                eh = sb.tile([128, N], DT_MM, tag="eh")
                nc.vector.scalar_tensor_tensor(
                    eh[:], e1b[:, h * N:(h + 1) * N],
                    f1[:, jc * 8 + h:jc * 8 + h + 1], bt[:], ALU.mult, ALU.max)
                # accumulate [g|1].T @ P.T -> o4[32hh .. +17, :]
                lhsT = g_ext[:, jc * 136 + h * 17: jc * 136 + (h + 1) * 17]
                for ih in range(2):
                    nc.tensor.matmul(
                        o4[32 * hh:32 * hh + 17, ih * 512:(ih + 1) * 512],
                        lhsT, eh[:, ih * 512:(ih + 1) * 512],
                        start=(jc == 0), stop=(jc == NC128 - 1),
                        tile_position=(0, 32 * hh))

        # psum -> sbuf (per valid partition range), then shift each head's
        # [17, 1024] block down to base partition 0 via SBUF->SBUF DMA
        # (matmuls from non-zero base partitions crash here, DMA can't read PSUM)
        o4s = sb.tile([128, N], FP32, tag="o4s")
        for hh in range(4):
            nc.scalar.copy(o4s[32 * hh:32 * hh + 17, :],
                           o4[32 * hh:32 * hh + 17, :])
        ohs = sb.tile([17, 4 * N], FP32, tag="ohs")
        for hh in range(4):
            nc.sync.dma_start(ohs[:, hh * N:(hh + 1) * N],
                              o4s[32 * hh:32 * hh + 17, :])

        # transpose back per i-chunk, normalize (relu folded into final op)
        for icn in range(NC128):
            t4 = ps_t4.tile([128, 4 * 17], FP32, tag="t4")
            for hh in range(4):
                nc.tensor.transpose(
                    t4[:, hh * 17:(hh + 1) * 17],
                    ohs[:, hh * N + icn * 128: hh * N + (icn + 1) * 128],
                    ident[0:17, 0:17])
            r4 = sb.tile([128, 4], FP32, tag="r4")
            den = t4[:].rearrange("p (a q) -> p a q", q=17)[:, :, 16:17]
            nc.vector.reciprocal(r4[:].rearrange("p (a q) -> p a q", q=1), den)
            for hh in range(4):
                h = grp * 4 + hh
                # out = max(num * (1/den), 0) == relu(num)/den  (den > 0)
                nc.vector.tensor_scalar(
                    outt[icn][:, h * HD:(h + 1) * HD],
                    t4[:, hh * 17:hh * 17 + 16],
                    r4[:, hh:hh + 1], 0.0, ALU.mult, ALU.max)

    for icn in range(NC128):
        nc.sync.dma_start(out_d.ap()[icn * 128:(icn + 1) * 128, :], outt[icn][:])


# ---- host wrapper ----
_CACHE = {}


def _prep_weights(W, Wa):
    W = np.asarray(W, dtype=np.float32)
    Wa = np.asarray(Wa, dtype=np.float32)
    wall = np.ascontiguousarray(W.transpose(1, 0, 2).reshape(DI, H * HD))
    wabd = np.zeros((DI, 2 * H), dtype=np.float32)
    for hh in range(H):
        wabd[hh * HD:(hh + 1) * HD, hh] = Wa[hh, :HD]
        wabd[hh * HD:(hh + 1) * HD, H + hh] = Wa[hh, HD:]
    ident = np.eye(128, dtype=np.float32)
    ident4 = np.zeros((128, 32), dtype=np.float32)
    for a in range(4):
        for k in range(32):
            ident4[32 * a + k, k] = 1.0
    return wall, wabd, ident, ident4


def kernel(h, W, Wa):
    h = np.asarray(h, dtype=np.float32)
    if "nc" not in _CACHE:
        _CACHE["nc"] = build_nc(iters=1)
    nc = _CACHE["nc"]
    wall, wabd, ident, ident4 = _prep_weights(W, Wa)
    in_maps = [
        {"hb": np.ascontiguousarray(h[c]), "wall": wall, "wabd": wabd,
         "ident": ident, "ident4": ident4}
        for c in range(B)
    ]
    res = bass_utils.run_bass_kernel_spmd(nc, in_maps, core_ids=list(range(B)))
    out = np.stack([res.results[c]["out"] for c in range(B)], axis=0)
    return out.astype(np.float32)


# revision 26
# speedup vs baseline: 34.1894x; 34.1894x over previous
"""GAT multi-head attention kernel for Trainium2 (8 NeuronCores, data-parallel over batch).

Problem (hardcoded): h [8,1024,128] f32, W [8,128,16] f32, Wa [8,32] f32.
  g   = einsum('bni,hid->hbnd', h, W)
  e   = leakyrelu(g@a_src [i] + g@a_dst [j], 0.2)      # [H,B,N,N]
  P   = softmax_j(e)
  out = relu(einsum('hbij,hbjd->bihd', P, g)).reshape(B,N,H*hd)

Sharding: graph b -> core b. Each core computes all 8 heads for its graph.

Algebra: with s=g@a_src (per-i), t=g@a_dst (per-j),
  exp(leakyrelu(s_i+t_j)) = max(e^{s_i+t_j}, e^{.2(s_i+t_j)})
                          = e^{.2 s_i} * e^{t_j} * max(e^{.8 s_i}, e^{-.8 t_j})
The e^{.2 s_i} factor is constant along the softmax axis j, so it cancels in
the softmax ratio and is simply dropped. The e^{t_j} factor rides along the
contraction dim of the attention matmul and is folded into the stationary
[g|1] (whose ones column also yields the softmax denominator). Each NxN
probability tile then costs ONE single-input op:
  u_ij = max(q_i, r_j),  q = e^{.8 s} (broadcast rows), r = e^{-.8 t} (col).
computed as tensor_scalar(max) on DVE or GPSIMD.
"""
import numpy as np
from contextlib import ExitStack

import concourse.bass as bass
import concourse.tile as tile
from concourse import bacc, mybir
from concourse import bass_utils

# ---- problem constants (from spec; kernel.py must be self-contained) ----
B, N, DI, H, HD = 8, 1024, 128, 8, 16
SLOPE = 0.2
NC128 = N // 128            # 8 chunks of 128
FP32 = mybir.dt.float32
BF16 = mybir.dt.bfloat16

AF = mybir.ActivationFunctionType
ALU = mybir.AluOpType

DT_MM = BF16          # dtype of probability tiles + attention-matmul operands

# ---- engine routing knobs ----
GPS_JCS = ()    # jc values whose u-tiles run on GPSIMD (never: ~10x slower)
ACT_JCS = ()    # jc values whose u-tiles run on the scalar engine (off: 2-op chain scheduled worse)


def build_nc(iters: int = 1, variant: str = "full"):
    nc = bacc.Bacc("TRN2", target_bir_lowering=False, debug=False, num_devices=8)

    hb_d = nc.dram_tensor("hb", [N, DI], FP32, kind="ExternalInput")
    wall_d = nc.dram_tensor("wall", [DI, H * HD], FP32, kind="ExternalInput")
    wabd_d = nc.dram_tensor("wabd", [DI, 2 * H], FP32, kind="ExternalInput")
    ident_d = nc.dram_tensor("ident", [128, 128], FP32, kind="ExternalInput")
    out_d = nc.dram_tensor("out", [N, H * HD], FP32, kind="ExternalOutput")

    with tile.TileContext(nc) as tc:
        with ExitStack() as ctx:
            if iters > 1:
                ctx.enter_context(tc.For_i(
                    0, iters, 1,
                    hint_engines=(mybir.EngineType.PE, mybir.EngineType.DVE,
                                  mybir.EngineType.Activation,
                                  mybir.EngineType.SP)))
            _body(ctx, tc, hb_d, wall_d, wabd_d, ident_d, out_d, variant)
    nc.compile()
    return nc


def _body(ctx, tc, hb_d, wall_d, wabd_d, ident_d, out_d, variant="full"):
    nc = tc.nc
    consts = ctx.enter_context(tc.tile_pool(name="consts", bufs=1))
    sb = ctx.enter_context(tc.tile_pool(name="sb", bufs=4))
    mtp = ctx.enter_context(tc.tile_pool(name="mtp", bufs=12))
    ps_small = ctx.enter_context(tc.tile_pool(name="ps_small", bufs=4, space="PSUM"))
    ps_oh = ctx.enter_context(tc.tile_pool(name="ps_oh", bufs=2, space="PSUM"))
    dram = ctx.enter_context(tc.tile_pool(name="dram", bufs=1, space="DRAM"))

    # ---- constants in ----
    ident = consts.tile([128, 128], FP32)
    nc.sync.dma_start(ident[:], ident_d.ap())
    wall = consts.tile([128, H * HD], FP32)
    nc.sync.dma_start(wall[:], wall_d.ap())
    wq = consts.tile([128, 2 * H], FP32)
    nc.sync.dma_start(wq[:], wabd_d.ap())

    # ---- phase A: load h (split DMAs), transpose to hT [128 i, 1024 n] ----
    hall = consts.tile([128, N], FP32)   # [p, c*128+i] = hb[c*128+p, i]
    for half in range(2):
        nc.sync.dma_start(
            hall[:, half * 512:(half + 1) * 512].rearrange(
                "p (c i) -> p c i", i=128),
            hb_d.ap()[half * 512:(half + 1) * 512, :].rearrange(
                "(c p) i -> p c i", p=128))
    hT = consts.tile([128, N], FP32)
    for icn in range(NC128):
        pt = ps_small.tile([128, 128], FP32, tag="ps", padded_shape=[128, 512])
        nc.tensor.transpose(pt[:], hall[:, icn * 128:(icn + 1) * 128], ident[:])
        nc.scalar.copy(hT[:, icn * 128:(icn + 1) * 128], pt[:])

    # ---- phase B: s rows straight from hT (wq = wall @ wabd, host-side),
    # then the broadcast chain, launched as early as possible ----
    srows = consts.tile([8, N], FP32)           # s_h(i) as rows
    for half in range(2):
        ps = ps_small.tile([8, 512], FP32, tag="ps", padded_shape=[128, 512])
        nc.tensor.matmul(ps[:], wq[:, 0:8], hT[:, half * 512:(half + 1) * 512],
                         start=True, stop=True)
        nc.scalar.copy(srows[:, half * 512:(half + 1) * 512], ps[:])

    # q rows = e^{0.8 s} -> bf16, bounce via DRAM, broadcast across partitions
    qrows = consts.tile([8, N], DT_MM)
    nc.scalar.activation(qrows[:], srows[:], AF.Exp, scale=0.8)
    qrows_d = dram.tile([H, N], DT_MM)
    nc.sync.dma_start(qrows_d[:], qrows[:])
    qb = consts.tile([128, H * N], DT_MM)
    for h in range(H):
        nc.sync.dma_start(qb[:, h * N:(h + 1) * N],
                          qrows_d[h:h + 1, :].partition_broadcast(128))

    # ---- phase C: st [128 n, jc*16 + (s_h | 8+t_h)] from hT, exp factors ----
    st = consts.tile([128, NC128 * 16], FP32)
    for jc in range(NC128):
        ps = ps_small.tile([128, 16], FP32, tag="ps", padded_shape=[128, 512])
        nc.tensor.matmul(ps[:], hT[:, jc * 128:(jc + 1) * 128], wq[:],
                         start=True, stop=True)
        nc.scalar.copy(st[:, jc * 16:(jc + 1) * 16], ps[:])

    t_view = st[:].rearrange("p (c q) -> p c q", q=16)[:, :, 8:16]
    # f1 = e^t (moving-side scaling), rcols = e^{-0.8 t} (u-tile scalar)
    f1 = consts.tile([128, NC128 * 8], FP32)
    nc.scalar.activation(f1[:].rearrange("p (c q) -> p c q", q=8), t_view, AF.Exp)
    rcols = consts.tile([128, NC128 * 8], FP32)
    nc.scalar.activation(rcols[:].rearrange("p (c q) -> p c q", q=8), t_view,
                         AF.Exp, scale=-0.8)
    f1r = consts.tile([128, NC128 * 8], FP32)   # e^{0.2 t} = f1 * rcols
    nc.scalar.activation(f1r[:].rearrange("p (c q) -> p c q", q=8), t_view,
                         AF.Exp, scale=SLOPE)
    rneg = consts.tile([128, NC128 * 8], FP32)  # -e^{-0.8 t}
    nc.vector.tensor_scalar(rneg[:], rcols[:], -1.0, None, ALU.mult)

    # ---- phase D: g_ext [128 j, jc*256 + h*32 + d]; col 16 = ones (den),
    # cols 17..31 zero padding so matmuls cover all 128 psum partitions ----
    g_ext = consts.tile([128, NC128 * 256], DT_MM)
    nc.vector.memset(g_ext[:], 0.0)
    ones_view = g_ext[:].rearrange("p (c q) -> p c q", q=32)[:, :, 16:17]
    nc.vector.memset(ones_view, 1.0)
    for jc in range(NC128):
        ps = ps_small.tile([128, 128], FP32, tag="ps", padded_shape=[128, 512])
        nc.tensor.matmul(ps[:], hT[:, jc * 128:(jc + 1) * 128], wall[:],
                         start=True, stop=True)
        dst = g_ext[:, jc * 256:(jc + 1) * 256].rearrange(
            "p (h q) -> p h q", q=32)[:, :, 0:16]
        src = ps[:].rearrange("p (h q) -> p h q", q=16)
        nc.scalar.copy(dst, src)

    # ---- phase E: u tiles + attention matmuls (4 heads packed per PSUM
    # accumulator at partition offsets 32*hh via col-group tile_position) ----
    o4s = [consts.tile([128, N], FP32, name=f"o4s{g}") for g in range(2)]
    outt_all = consts.tile([128, NC128 * H * HD], FP32)
    outt = [outt_all[:, i * H * HD:(i + 1) * H * HD] for i in range(NC128)]

    if variant == "ad":
        nc.vector.memset(outt_all[:], 0.0)
        nc.sync.dma_start(
            out_d.ap().rearrange("(c p) i -> p c i", p=128),
            outt_all[:].rearrange("p (c i) -> p c i", i=H * HD))
        return

    if variant == "ss":
        # scale stationary in place by e^t on ACT (incl. ones col)
        for jc in range(NC128):
            for h in range(H):
                sl = slice(jc * 256 + h * 32, jc * 256 + h * 32 + 17)
                nc.scalar.activation(g_ext[:, sl], g_ext[:, sl], AF.Copy,
                                     scale=f1[:, jc * 8 + h:jc * 8 + h + 1])

    for grp in range(2):
        o4 = ps_oh.tile([128, N], FP32, tag="oh")
        mts_shared = None
        if variant == "nodve":
            mts_shared = []
            for hh in range(4):
                h = grp * 4 + hh
                mt = mtp.tile([128, N], DT_MM, tag="mt", name=f"mts{hh}")
                nc.vector.tensor_scalar(mt[:], qb[:, h * N:(h + 1) * N],
                                        rcols[:, h:h + 1], f1[:, h:h + 1],
                                        ALU.max, ALU.mult)
                mts_shared.append(mt)
        for jc in range(NC128):
            if variant == "nodve":
                mts = mts_shared
            else:
                mts = []
                for hh in range(4):
                    h = grp * 4 + hh
                    mt = mtp.tile([128, N], DT_MM, tag="mt", name=f"mt{hh}")
                    c = jc * 8 + h
                    if jc in ACT_JCS and variant == "full":
                        # mt = F1*(relu(q - r) + r) = F1*max(q, r) on ACT
                        vt = mtp.tile([128, N], DT_MM, tag="vt", name=f"vt{hh}",
                                      bufs=6)
                        nc.scalar.activation(vt[:], qb[:, h * N:(h + 1) * N],
                                             AF.Relu, bias=rneg[:, c:c + 1])
                        nc.scalar.activation(mt[:], vt[:], AF.Identity,
                                             scale=f1[:, c:c + 1],
                                             bias=f1r[:, c:c + 1])
                    elif variant == "ss":
                        nc.vector.tensor_scalar(mt[:], qb[:, h * N:(h + 1) * N],
                                                rcols[:, c:c + 1], None, ALU.max)
                    else:
                        nc.vector.tensor_scalar(mt[:], qb[:, h * N:(h + 1) * N],
                                                rcols[:, c:c + 1],
                                                f1[:, c:c + 1],
                                                ALU.max, ALU.mult)
                    mts.append(mt)
            # 4 heads' matmuls back-to-back at 4 col-groups -> they stream
            # concurrently on separate XBUSes (col-tiling concurrency)
            for ih in range(2):
                for hh in range(4):
                    h = grp * 4 + hh
                    lhsT = g_ext[:, jc * 256 + h * 32: jc * 256 + (h + 1) * 32]
                    nc.tensor.matmul(
                        o4[32 * hh:32 * hh + 32, ih * 512:(ih + 1) * 512],
                        lhsT, mts[hh][:, ih * 512:(ih + 1) * 512],
                        start=(jc == 0), stop=(jc == NC128 - 1),
                        tile_position=(0, 32 * hh), skip_group_check=True)
        for ih in range(2):
            nc.scalar.copy(o4s[grp][:, ih * 512:(ih + 1) * 512],
                           o4[:, ih * 512:(ih + 1) * 512])

    # ---- phase F: one [128,128] transpose per (grp, i-chunk), normalize ----
    for grp in range(2):
        for icn in range(NC128):
            t4 = ps_small.tile([128, 128], FP32, tag="ps",
                               padded_shape=[128, 512])
            nc.tensor.transpose(
                t4[:], o4s[grp][:, icn * 128:(icn + 1) * 128], ident[:])
            r4 = sb.tile([128, 4], FP32, tag="r4")
            den = t4[:].rearrange("p (a q) -> p a q", q=32)[:, :, 16:17]
            nc.vector.reciprocal(r4[:].rearrange("p (a q) -> p a q", q=1), den)
            for hh in range(4):
                h = grp * 4 + hh
                # relu(r*num) == relu(num)/den since den>0
                nc.scalar.activation(outt[icn][:, h * HD:(h + 1) * HD],
                                     t4[:, 32 * hh:32 * hh + 16], AF.Relu,
                                     scale=r4[:, hh:hh + 1])

    nc.sync.dma_start(
        out_d.ap().rearrange("(c p) i -> p c i", p=128),
        outt_all[:].rearrange("p (c i) -> p c i", i=H * HD))


# ---- host wrapper ----
_CACHE = {}


def _prep_weights(W, Wa):
    W = np.asarray(W, dtype=np.float32)
    Wa = np.asarray(Wa, dtype=np.float32)
    wall = np.ascontiguousarray(W.transpose(1, 0, 2).reshape(DI, H * HD))
    wabd = np.zeros((DI, 2 * H), dtype=np.float32)
    for hh in range(H):
        wabd[hh * HD:(hh + 1) * HD, hh] = Wa[hh, :HD]
        wabd[hh * HD:(hh + 1) * HD, H + hh] = Wa[hh, HD:]
    wq = np.ascontiguousarray(wall @ wabd)   # s/t projections direct from hT
    ident = np.eye(128, dtype=np.float32)
    return wall, wq, ident


def kernel(h, W, Wa):
    h = np.asarray(h, dtype=np.float32)
    if "nc" not in _CACHE:
        _CACHE["nc"] = build_nc(iters=1)
    nc = _CACHE["nc"]
    wall, wabd, ident = _prep_weights(W, Wa)
    in_maps = [
        {"hb": np.ascontiguousarray(h[c]), "wall": wall, "wabd": wabd,
         "ident": ident}
        for c in range(B)
    ]
    res = bass_utils.run_bass_kernel_spmd(nc, in_maps, core_ids=list(range(B)))
    out = np.stack([res.results[c]["out"] for c in range(B)], axis=0)
    return out.astype(np.float32)
